# revision 44
# baseline (speedup 1.0000x reference)
"""Trainium2 Bass kernel for nn_MembraneLayer: h = x @ w followed by a
double first-order recurrence over time, producing (syn_rec, mem_rec).

Sharding: data-parallel over batch. 8 cores x 64 batches each.

Per-core layout (hardcoded):
  xt_p [128, 38400] fp16  x^T packed per (quartet, group) chunk: col =
                          q*9600 + g*2400 + k*400 + (b*100 + t); contraction
                          rows zero-padded to 768 (k=5 tile: rows 640..699).
  w_p  [128, 3072]  fp16  w packed: col = di*768 + k*128 + c.
  ab   [128, 8]     f32   cols 0-3 alpha per di, cols 4-7 beta per di.
  out2 [128, 2, 4, 6400] fp16  plane 0 = syn_rec, plane 1 = v (= mem/(1-beta)
                          shifted); [p, plane, di, q*1600 + g*400 + b*100 + t].

Engine split per (q, di) unit: the PE accumulates h into four per-bank
PSUM tiles (shifted one step: h[b,t] lands at col t+1, g-outer/k-inner
so each bank unblocks downstream as soon as its x chunk lands). The
Scalar (ACT) engine drains the four banks into one SBUF staging tile so
the DVE runs ONE merged fp32 alpha-scan -> fp16 syn, then one beta-scan
over syn -> fp16 v (DVE scans are the 2cyc/elem bottleneck, ~112us, and
run back-to-back). The first unit scans PSUM per-bank directly so the
scan pipeline starts right after bank 0's six matmuls. Output DMAs ride
the gpsimd SWDGE queue (x owns sync, ACT copies own scalar) to avoid
head-of-line blocking; the last unit's v-scan is split into quarters on
the scalar queue to shorten the tail. Coefficient tiles are built on
device (ACT broadcast + GPSIMD strided zeros); coefs are zero at t=0,1
of each series so the stale PSUM col-0 junk never propagates (host
zeroes syn[t=0] and mem[t<2], which are exactly zero in the reference).
The (1-beta) scale on mem is applied on the host during unshard; 16
warmup matmuls pre-fill all PSUM banks with finite values and raise the
PE HAM clock gate before the first real matmul.
"""

import os

import numpy as np

import concourse.bass as bass
import concourse.tile as tile
from concourse import bacc, mybir
from concourse import bass_utils

B, T, C, D = 512, 100, 700, 512
NCORES = 8
BC = B // NCORES  # 64 batches per core
CP = 768  # padded contraction
F32 = mybir.dt.float32
FP16 = mybir.dt.float16

MODE = "fp16-scan2"  # informational (printed by test.py)

LAST_RESULT = None
_cache = {}


def _build():
    if "nc" in _cache:
        return _cache["nc"]
    nc = bacc.Bacc("TRN2", target_bir_lowering=False, debug=False)

    xt_d = nc.dram_tensor("xt_p", [128, 38400], FP16, kind="ExternalInput").ap()
    w_d = nc.dram_tensor("w_p", [128, 3072], FP16, kind="ExternalInput").ap()
    ab_d = nc.dram_tensor("ab", [128, 8], F32, kind="ExternalInput").ap()
    out_d = nc.dram_tensor("out2", [128, 2, 4, 6400], FP16, kind="ExternalOutput").ap()

    MUL = mybir.AluOpType.mult
    ADD = mybir.AluOpType.add

    with tile.TileContext(nc) as tc:
        from contextlib import ExitStack

        with ExitStack() as ctx:
            cpool = ctx.enter_context(tc.tile_pool(name="consts", bufs=1))
            # ab rides the (initially idle) scalar HWDGE queue; w is split
            # per-k at the head of the sync queue so the first matmuls are
            # unblocked within ~1us instead of waiting for one 786KB DMA
            # that crawls behind the x stream
            ab_sb = cpool.tile([128, 8], F32, name="ab_sb", tag="ab")
            nc.scalar.dma_start(ab_sb[:], ab_d)
            wd = []
            for di in range(4):
                t_ = cpool.tile([128, 768], FP16, name=f"wd{di}", tag=f"wd{di}")
                nc.sync.dma_start(t_[:], w_d[:, di * 768 : (di + 1) * 768])
                wd.append(t_)

            # coef tiles built on device: ACT broadcasts alpha/beta along the
            # free dim, GPSIMD zeroes t=0,1 of each 100-step series. Tiles
            # are di-PAIR sized so q1/q2 can run pair-merged scans; builds
            # for di>=2 are emitted later so they don't contend with the
            # first units' PSUM-drain copies on the ACT queue.
            ones = cpool.tile([128, 1600], F32, name="ones", tag="ones")
            nc.gpsimd.memset(ones[:], 1.0)
            # zeros at t=0,1 baked into the pattern once: every coef mul
            # inherits them, dropping 8 per-coef memsets and their
            # cross-engine latency chains
            nc.gpsimd.memset(
                ones[:].rearrange("p (b t) -> p b t", t=100)[:, :, 0:2], 0.0
            )
            acp = [
                cpool.tile([128, 3200], F32, name=f"acp{p}", tag=f"acp{p}")
                for p in range(2)
            ]
            bcp = [
                cpool.tile([128, 3200], F32, name=f"bcp{p}", tag=f"bcp{p}")
                for p in range(2)
            ]
            ac_t = [acp[i // 2][:, (i % 2) * 1600 : (i % 2 + 1) * 1600] for i in range(4)]
            bc_t = [bcp[i // 2][:, (i % 2) * 1600 : (i % 2 + 1) * 1600] for i in range(4)]

            def emit_coef(i):
                nc.scalar.mul(ac_t[i], ones[:], ab_sb[:, i : i + 1])
                nc.scalar.mul(bc_t[i], ones[:], ab_sb[:, 4 + i : 5 + i])

            for i in range(4):
                emit_coef(i)

            xp = ctx.enter_context(tc.tile_pool(name="xp", bufs=2))
            pp = ctx.enter_context(tc.tile_pool(name="pp", bufs=2, space="PSUM"))
            svp = ctx.enter_context(tc.tile_pool(name="svp", bufs=3))
            hp = ctx.enter_context(tc.tile_pool(name="hp", bufs=3))
            sv2p = ctx.enter_context(tc.tile_pool(name="sv2p", bufs=2))
            hp2 = ctx.enter_context(tc.tile_pool(name="hp2", bufs=2))

            # warmup: fills every PSUM bank with finite values (the scans read
            # one stale col per series) and un-throttles the PE HAM before the
            # first real matmul
            warm = cpool.tile([128, 512], FP16, name="warm", tag="warm")
            nc.vector.memset(warm[:], 0.0)
            for _ in range(2):
                for g in range(4):
                    ps_w = pp.tile([128, 512], F32, tag=f"ps{g}", name=f"warm_ps{g}")
                    nc.tensor.matmul(
                        ps_w[:], warm[:, 0:128], warm[:], start=True, stop=True
                    )

            x00 = [
                xp.tile([128, 1200], FP16, tag=f"x00{h}", name=f"x00{h}")
                for h in range(2)
            ]
            nc.scalar.dma_start(x00[0][:], xt_d[:, 0:1200])
            nc.scalar.dma_start(x00[1][:], xt_d[:, 1200:2400])
            for q in range(4):
                xc = []
                for g in range(4):
                    if q == 0 and g == 0:
                        xc.append(None)  # uses the split x00 tiles
                        continue
                    t_ = xp.tile([128, 2400], FP16, tag=f"xc{g}", name=f"xc{q}_{g}")
                    base = q * 9600 + g * 2400
                    nc.sync.dma_start(t_[:], xt_d[:, base : base + 2400])
                    xc.append(t_)
                def unit_mms(di):
                    # g outer / k inner over per-bank PSUM tiles: each bank
                    # finishes as soon as its x chunk lands, and downstream
                    # consumers depend on just that bank's 6 matmuls
                    psg = []
                    for g in range(4):
                        ps = pp.tile(
                            [128, 512], F32, tag=f"ps{g}", name=f"ps_{q}_{di}_{g}"
                        )
                        psg.append(ps)
                        out3 = ps[:, 0:400].rearrange("p (b t) -> p b t", t=100)[
                            :, :, 1:100
                        ]
                        for k in range(6):
                            lhsT = wd[di][:, k * 128 : (k + 1) * 128]
                            if q == 0 and g == 0:
                                xt_src = x00[k // 3][
                                    :, (k % 3) * 400 : (k % 3) * 400 + 400
                                ]
                            else:
                                xt_src = xc[g][:, k * 400 : (k + 1) * 400]
                            rhs3 = xt_src.rearrange("p (b t) -> p b t", t=100)[
                                :, :, 0:99
                            ]
                            nc.tensor.matmul(
                                out3, lhsT, rhs3, start=(k == 0), stop=(k == 5)
                            )
                    return psg

                if q in (1, 2, 3):
                    # di-PAIR merged path: one FD=3200 syn scan and one
                    # FD=3200 v scan per pair of units (halves the scan
                    # fixed costs and boundary overheads); q3 pairs only
                    # di 0-1 so the kernel tail ends on small scans
                    for dp in range(2) if q in (1, 2) else range(1):
                        hs2 = hp2.tile(
                            [128, 3200], F32, tag="hs2", name=f"hs2_{q}_{dp}"
                        )
                        for du in range(2):
                            psg = unit_mms(2 * dp + du)
                            for g in range(4):
                                nc.scalar.copy(
                                    hs2[
                                        :,
                                        du * 1600 + g * 400 : du * 1600 + (g + 1) * 400,
                                    ],
                                    psg[g][:, 0:400],
                                )
                        sv2 = sv2p.tile(
                            [128, 6400], FP16, tag="sv2", name=f"sv2_{q}_{dp}"
                        )
                        nc.vector.tensor_tensor_scan(
                            sv2[:, 0:3200], acp[dp][:], hs2[:], 0.0, MUL, ADD
                        )
                        nc.gpsimd.dma_start(
                            out_d[:, 0, 2 * dp : 2 * dp + 2, q * 1600 : (q + 1) * 1600],
                            sv2[:, 0:3200].rearrange("p (d c) -> p d c", c=1600),
                        )
                        nc.vector.tensor_tensor_scan(
                            sv2[:, 3200:6400], bcp[dp][:], sv2[:, 0:3200], 0.0, MUL, ADD
                        )
                        nc.gpsimd.dma_start(
                            out_d[:, 1, 2 * dp : 2 * dp + 2, q * 1600 : (q + 1) * 1600],
                            sv2[:, 3200:6400].rearrange("p (d c) -> p d c", c=1600),
                        )
                    if q != 3:
                        continue

                for di in range(4) if q == 0 else (2, 3):
                    psg = unit_mms(di)
                    sv = svp.tile([128, 3200], FP16, tag="sv", name=f"sv_{q}_{di}")
                    if q == 0 and di == 0:
                        # direct per-bank scans, with the per-bank v scan
                        # interleaved: banks are whole batches, so the v
                        # recurrence is independent per bank and the DVE has
                        # work while the PE fills the next bank
                        for g in range(4):
                            nc.vector.tensor_tensor_scan(
                                sv[:, g * 400 : (g + 1) * 400],
                                ac_t[di][:, 0:400],
                                psg[g][:, 0:400],
                                0.0,
                                MUL,
                                ADD,
                            )
                            nc.vector.tensor_tensor_scan(
                                sv[:, 1600 + g * 400 : 1600 + (g + 1) * 400],
                                bc_t[di][:, 0:400],
                                sv[:, g * 400 : (g + 1) * 400],
                                0.0,
                                MUL,
                                ADD,
                            )
                    else:
                        # ACT drains PSUM into an SBUF staging tile so DVE
                        # runs one merged syn scan (drops 3 scan fixed costs
                        # and the PSUM access latency)
                        hs = hp.tile([128, 1600], F32, tag="hs", name=f"hs_{q}_{di}")
                        for g in range(4):
                            nc.scalar.copy(
                                hs[:, g * 400 : (g + 1) * 400], psg[g][:, 0:400]
                            )
                        nc.vector.tensor_tensor_scan(
                            sv[:, 0:1600], ac_t[di][:], hs[:], 0.0, MUL, ADD
                        )
                    oeng = nc.scalar if (q == 3 and di == 3) else nc.gpsimd
                    oeng.dma_start(
                        out_d[:, 0, di, q * 1600 : (q + 1) * 1600],
                        sv[:, 0:1600],
                    )
                    if q == 0 and di == 0:
                        # v already computed bank-by-bank above
                        nc.gpsimd.dma_start(
                            out_d[:, 1, di, q * 1600 : (q + 1) * 1600],
                            sv[:, 1600:3200],
                        )
                    elif q == 3 and di == 3:
                        # split the final v scan so each quarter's DMA
                        # overlaps the next quarter (shorter kernel tail)
                        for h in range(4):
                            nc.vector.tensor_tensor_scan(
                                sv[:, 1600 + h * 400 : 2000 + h * 400],
                                bc_t[di][:, h * 400 : (h + 1) * 400],
                                sv[:, h * 400 : (h + 1) * 400],
                                0.0,
                                MUL,
                                ADD,
                            )
                            nc.scalar.dma_start(
                                out_d[
                                    :, 1, di, q * 1600 + h * 400 : q * 1600 + (h + 1) * 400
                                ],
                                sv[:, 1600 + h * 400 : 2000 + h * 400],
                            )
                    else:
                        nc.vector.tensor_tensor_scan(
                            sv[:, 1600:3200],
                            bc_t[di][:],
                            sv[:, 0:1600],
                            0.0,
                            MUL,
                            ADD,
                        )
                        nc.gpsimd.dma_start(
                            out_d[:, 1, di, q * 1600 : (q + 1) * 1600],
                            sv[:, 1600:3200],
                        )

    nc.compile()
    _cache["nc"] = nc
    return nc


def kernel(inputs, w, alpha, beta):
    global LAST_RESULT
    inputs = np.asarray(inputs, dtype=np.float32)
    w = np.asarray(w, dtype=np.float32)
    alpha = np.asarray(alpha, dtype=np.float32).reshape(-1)
    beta = np.asarray(beta, dtype=np.float32).reshape(-1)

    nc = _build()

    w_pad = np.zeros((CP, D), dtype=np.float32)
    w_pad[:C] = w
    # [128, di*768 + k*128 + c]
    w_p = np.ascontiguousarray(
        w_pad.reshape(6, 128, 4, 128).transpose(1, 2, 0, 3).reshape(128, 3072)
    ).astype(np.float16)
    ab = np.empty((128, 8), dtype=np.float32)
    for di in range(4):
        ab[:, di] = alpha[di * 128 : (di + 1) * 128]
        ab[:, 4 + di] = beta[di * 128 : (di + 1) * 128]

    in_maps = []
    for c in range(NCORES):
        xc = inputs[c * BC : (c + 1) * BC]  # [64, 100, 700]
        xt = np.zeros((CP, BC * T), dtype=np.float32)
        xt[:C] = xc.reshape(BC * T, C).T
        # [k, 128, q, g, 400] -> [128, q, g, k, 400] -> [128, 38400]
        xt_p = np.ascontiguousarray(
            xt.reshape(6, 128, 4, 4, 400).transpose(1, 2, 3, 0, 4).reshape(128, 38400)
        ).astype(np.float16)
        in_maps.append({"xt_p": xt_p, "w_p": w_p, "ab": ab})

    run_kwargs = {}
    if os.environ.get("MEMBRANE_TRACE_DIR"):
        run_kwargs["tmpdir"] = os.environ["MEMBRANE_TRACE_DIR"]
    res = bass_utils.run_bass_kernel_spmd(
        nc, in_maps, core_ids=list(range(NCORES)), **run_kwargs
    )
    LAST_RESULT = res

    omb = (1.0 - beta).reshape(1, 1, D)
    syn_full = np.empty((B, T, D), dtype=np.float32)
    mem_full = np.empty((B, T, D), dtype=np.float32)
    for c in range(NCORES):
        o = np.asarray(res.results[c]["out2"]).reshape(128, 2, 4, BC, T)
        syn_c = o[:, 0].astype(np.float32).transpose(2, 3, 1, 0).reshape(BC, T, D)
        v_c = o[:, 1].astype(np.float32).transpose(2, 3, 1, 0).reshape(BC, T, D)
        syn_full[c * BC : (c + 1) * BC] = syn_c
        mem_full[c * BC : (c + 1) * BC, 1:] = v_c[:, : T - 1] * omb
    syn_full[:, 0, :] = 0.0
    mem_full[:, 0:2, :] = 0.0  # mem[t=1] is exactly 0 in the reference
    return (syn_full, mem_full)
